# revision 3
# baseline (speedup 1.0000x reference)
"""Single-head causal attention (B=8, T=2048, H=1024, D=64) on 8 TRN2 NeuronCores.

Data-parallel over batch: one batch element per core, no collectives.

Per core, the sequence is processed as G=4 column groups of 512 so that
input DMA, projections, scores/AV and normalization pipeline with no
engine ever idling:

  Input xt bf16 pre-laid [G, 2, 64, hb, 512]: per (group, partition-half)
  one dma_start with 8KB contiguous runs per partition; halves issued on
  the sync and scalar rings in group order, so group g's 1MB lands ~3us
  after group g-1 while proj(g-1) runs.
  Weights pre-packed [128, 8, 192] ([Wk | Wq | Wv] per h-block, 1/8
  folded into Wq).
  Per group: packed projection [Wk | Wq] -> psum [128,512] (k rows 0..63,
  q rows 64..127), ONE [128,512] cast into kqT (both halves, full DVE
  width), q DMA-shifted to partitions 0..63 (gpsimd ring, hidden under
  the v projection); v proj -> vT cast -> 4 PE transposes into one psum
  tile -> strided copy into v_aug natural rows with a ones column.
  Scores sT[kj, qi] per 128-key block into a [128,512] psum bank; exp on
  ScalarE (no max subtraction: scores bounded ~+-4); diagonal blocks
  masked with gpsimd affine_select; pT bf16.
  AV: oT[d, qi] += [v[kb] | ones].T @ pT[kb] accumulating into ONE psum
  bank per group; the ones column accumulates the softmax denominator in
  row 64 for free.
  Normalize per group with no SBUF<->SBUF DMA hops: DVE reciprocal reads
  the psum sums row [1,512] directly, gpsimd partition_broadcast fans it
  to 64 partitions, DVE multiply, DMA out (sync ring).

The schedule is one flat software-pipelined stream: 12 PE warm-up
matmuls cover the first group's DMA (and ramp the HAM-gated PE clock),
then per group: proj, scores two ahead of AVs, with the NEXT group's
projection slotted before the diagonal-dependent AVs so the PE never
stalls on the exp chain and the clock never drops.  PSUM: 2 proj acc
banks, 3 score banks, 2 oT banks, 1 v-transpose bank = 8.
"""

import sys
from contextlib import ExitStack

if "/opt/trn_rl_repo" not in sys.path:
    sys.path.insert(0, "/opt/trn_rl_repo")

import numpy as np
import ml_dtypes

import concourse.bass as bass
import concourse.tile as tile
from concourse import bacc, mybir
from concourse.bass_utils import run_bass_kernel_spmd

B, T, H, D = 8, 2048, 1024, 64
N_CORES = 8
HB = H // 128  # 8 h-blocks
G = 4  # 512-wide column groups
GW = T // G  # 512
KB = T // 128  # 16 key blocks
KPG = KB // G  # 4 key blocks per group

LINEARIZE = False
F32 = mybir.dt.float32
BF16 = mybir.dt.bfloat16


def build_kernel():
    nc = bacc.Bacc("TRN2", target_bir_lowering=False, debug=False, num_devices=N_CORES)

    # xt: [group, partition-half, 64, h-block, t]; w: [128, hb, 192] = [Wk|Wq|Wv]
    xt_d = nc.dram_tensor("xt", [G, 2, 64, HB, GW], BF16, kind="ExternalInput").ap()
    w_d = nc.dram_tensor("w", [128, HB, 3 * D], BF16, kind="ExternalInput").ap()
    out_d = nc.dram_tensor("out", [D, T], F32, kind="ExternalOutput").ap()

    with tile.TileContext(nc, linearize=LINEARIZE) as tc:
        _build(tc, xt_d, w_d, out_d)

    nc.compile()
    return nc


def _build(tc, xt_d, w_d, out_d):
    nc = tc.nc
    ctx = ExitStack()
    singles = ctx.enter_context(tc.tile_pool(name="singles", bufs=1))
    pspool = ctx.enter_context(tc.tile_pool(name="pspool", bufs=2, space="PSUM"))
    scpool = ctx.enter_context(tc.tile_pool(name="scpool", bufs=3, space="PSUM"))
    opool = ctx.enter_context(tc.tile_pool(name="opool", bufs=2, space="PSUM"))
    vtpool = ctx.enter_context(tc.tile_pool(name="vtpool", bufs=1, space="PSUM"))
    ppool = ctx.enter_context(tc.tile_pool(name="ppool", bufs=4))

    # ---- input DMAs: group-ordered, each group split across two rings ----
    w_s = singles.tile([128, HB, 3 * D], BF16)
    xt_s = singles.tile([128, G, HB, GW], BF16)
    wu_s = singles.tile([128, 512], BF16, name="wu_s")
    nc.vector.memset(wu_s[:], 0.0)
    nc.gpsimd.dma_start(out=w_s[:], in_=w_d[:])
    for g in range(G):
        nc.sync.dma_start(
            out=xt_s[0:64, g].rearrange("p hb t -> p (hb t)"),
            in_=xt_d[g, 0].rearrange("p hb t -> p (hb t)"),
        )
        nc.scalar.dma_start(
            out=xt_s[64:128, g].rearrange("p hb t -> p (hb t)"),
            in_=xt_d[g, 1].rearrange("p hb t -> p (hb t)"),
        )

    wkq = w_s[:, :, 0:128]  # [Wk | Wq] stationary halves
    wv = w_s[:, :, 128:192]

    kqT = singles.tile([128, T], BF16)  # rows 0..63 kT, rows 64..127 q
    qlo = singles.tile([64, T], BF16)  # q DMA-shifted to partitions 0..63
    vT = singles.tile([64, T], BF16)

    v_aug = singles.tile([128, KB, 65], BF16)
    nc.gpsimd.memset(v_aug[:, :, 64:65], 1.0)
    identb = singles.tile([64, 64], BF16)
    nc.gpsimd.memset(identb[:], 0.0)
    nc.gpsimd.affine_select(
        out=identb[:], in_=identb[:], compare_op=mybir.AluOpType.not_equal,
        fill=1.0, base=0, pattern=[[-1, 64]], channel_multiplier=1,
    )

    oT_s = singles.tile([64, T], F32)
    pt = {}  # (g, kb) -> bf16 tile
    oT_b = {}  # g -> [65, 512] psum tile

    def emit_warmup(n):
        wu_ps = pspool.tile([128, 512], F32, tag="ps", name="warmup")
        for _ in range(n):
            nc.tensor.matmul(
                wu_ps[:], wu_s[:, 0:128], wu_s[:], start=True, stop=True
            )

    def emit_proj_kq(g):
        gcols = bass.ds(g * GW, GW)
        acc = pspool.tile([128, GW], F32, tag="ps", name=f"acc_kq_{g}")
        for hb in range(HB):
            nc.tensor.matmul(
                acc[:],
                wkq[:, hb, :],
                xt_s[:, g, hb, :],
                start=(hb == 0),
                stop=(hb == HB - 1),
            )
        nc.vector.tensor_copy(kqT[:, gcols], acc[:])
        nc.gpsimd.dma_start(out=qlo[:, gcols], in_=kqT[64:128, gcols])

    def emit_proj_v(g):
        gcols = bass.ds(g * GW, GW)
        acc = pspool.tile([64, GW], F32, tag="ps", name=f"acc_v_{g}")
        for hb in range(HB):
            nc.tensor.matmul(
                acc[:],
                wv[:, hb, :],
                xt_s[:, g, hb, :],
                start=(hb == 0),
                stop=(hb == HB - 1),
            )
        nc.vector.tensor_copy(vT[:, gcols], acc[:])
        # natural v rows via PE transpose: 4 blocks into one psum tile,
        # then one strided copy into v_aug
        vtr = vtpool.tile([128, KPG, 64], BF16, tag="vt", name=f"vtr_{g}")
        for j in range(KPG):
            kb = KPG * g + j
            nc.tensor.transpose(
                vtr[:, j], vT[:, bass.ts(kb, 128)], identb[:]
            )
        nc.vector.tensor_copy(v_aug[:, KPG * g : KPG * (g + 1), 0:64], vtr[:])

    def emit_score(g, kb):
        # score block: kj in [kb*128, kb*128+128), qi in group g cols
        l0 = max(kb * 128 - g * GW, 0)  # group-local first column
        s_ps = scpool.tile([128, GW], F32, tag="sc", name=f"s_{g}_{kb}")
        nc.tensor.matmul(
            s_ps[:, l0:GW],
            kqT[0:64, bass.ts(kb, 128)],
            qlo[:, bass.ds(g * GW + l0, GW - l0)],
            start=True,
            stop=True,
        )
        p = ppool.tile([128, GW], BF16, tag="pt", name=f"pt_{g}_{kb}")
        pt[(g, kb)] = p
        nc.scalar.activation(
            out=p[:, l0:GW],
            in_=s_ps[:, l0:GW],
            func=mybir.ActivationFunctionType.Exp,
        )
        if kb >= KPG * g:
            # diagonal block: zero where kj (partition) > qi (free)
            nc.gpsimd.affine_select(
                out=p[:, l0 : l0 + 128],
                in_=p[:, l0 : l0 + 128],
                compare_op=mybir.AluOpType.is_ge,
                fill=0.0,
                base=0,
                pattern=[[1, 128]],
                channel_multiplier=-1,
            )

    def emit_av(g, kb):
        l0 = max(kb * 128 - g * GW, 0)
        if g not in oT_b:
            oT_b[g] = opool.tile([65, GW], F32, tag="o", name=f"oT_{g}")
        nc.tensor.matmul(
            oT_b[g][:, l0:GW],
            v_aug[:, kb, :],
            pt[(g, kb)][:, l0:GW],
            start=(kb == 0),
            stop=(kb == KPG * (g + 1) - 1),
        )

    def emit_norm(g):
        # normalize + store group g: psum sums row -> DVE reciprocal ->
        # gpsimd partition_broadcast -> DVE multiply -> DMA out.
        gcols = bass.ds(g * GW, GW)
        rcp = singles.tile([1, GW], F32, name=f"rcp_{g}")
        nc.vector.reciprocal(out=rcp[:], in_=oT_b[g][64:65, :])
        rb = singles.tile([64, GW], F32, name=f"rb_{g}")
        nc.gpsimd.partition_broadcast(rb[:], rcp[:])
        nc.vector.tensor_mul(oT_s[:, gcols], oT_b[g][0:64, :], rb[:])
        nc.sync.dma_start(out=out_d[:, gcols], in_=oT_s[:, gcols])

    # ---- flat schedule ----
    emit_warmup(12)
    emit_proj_kq(0)
    emit_proj_v(0)
    for g in range(G):
        last = KPG * (g + 1) - 1  # last key block of this group
        # scores run two ahead of AVs
        emit_score(g, 0)
        emit_score(g, 1)
        for kb in range(2, last + 1):
            emit_score(g, kb)
            emit_av(g, kb - 2)
        # next group's projection before the diagonal-dependent AVs
        if g + 1 < G:
            emit_av(g, last - 1)
            emit_proj_kq(g + 1)
            emit_av(g, last)
            emit_proj_v(g + 1)
        else:
            emit_av(g, last - 1)
            emit_av(g, last)
        emit_norm(g)

    ctx.close()


_NC_CACHE = {}


def _get_nc():
    if "nc" not in _NC_CACHE:
        _NC_CACHE["nc"] = build_kernel()
    return _NC_CACHE["nc"]


def make_in_maps(x, Wk, Wq, Wv):
    bf16 = ml_dtypes.bfloat16
    x = np.asarray(x, dtype=np.float32)
    wq = np.asarray(Wq, dtype=np.float32) / np.sqrt(np.float32(D))
    wk = np.asarray(Wk, dtype=np.float32)
    wv = np.asarray(Wv, dtype=np.float32)
    # [H, 192] = [Wk | Wq | Wv], then -> [128, HB, 192] (h = hb*128 + p)
    w = np.concatenate([wk, wq, wv], axis=1).astype(bf16)
    w = np.ascontiguousarray(w.reshape(HB, 128, 3 * D).transpose(1, 0, 2))
    in_maps = []
    for b in range(B):
        xt = x[b].T.astype(bf16)  # [H, T]
        # [G, 2, 64, HB, GW]: per (g, half, partition) one 8KB run
        xt = np.ascontiguousarray(
            xt.reshape(HB, 2, 64, G, GW).transpose(3, 1, 2, 0, 4)
        )
        in_maps.append({"xt": xt, "w": w})
    return in_maps


def kernel(x, Wk, Wq, Wv, **_ignored):
    nc = _get_nc()
    in_maps = make_in_maps(x, Wk, Wq, Wv)
    res = run_bass_kernel_spmd(nc, in_maps, core_ids=list(range(N_CORES)))
    out = np.stack([res.results[b]["out"].T for b in range(B)])
    return out.astype(np.float32)


if __name__ == "__main__":
    x = np.random.randn(B, T, H).astype(np.float32)
    s = 1.0 / np.sqrt(H)
    Wk = np.random.uniform(-s, s, (H, D)).astype(np.float32)
    Wq = np.random.uniform(-s, s, (H, D)).astype(np.float32)
    Wv = np.random.uniform(-s, s, (H, D)).astype(np.float32)
    out = kernel(x=x, Wk=Wk, Wq=Wq, Wv=Wv)
    print("out shape:", out.shape, "finite:", np.isfinite(out).all())


# revision 4
# speedup vs baseline: 1.1070x; 1.1070x over previous
"""Single-head causal attention (B=8, T=2048, H=1024, D=64) on 8 TRN2 NeuronCores.

Data-parallel over batch: one batch element per core, no collectives.

Per core, everything transposed so contractions land on partitions.
Input xt bf16 pre-laid [2(sg), 2(ph), 64, hb, 1024]: four dma_starts,
both rings (sync + scalar) stream super-group 0 concurrently (one
partition-half each, 16KB contiguous runs), then super-group 1 — sg0
lands ~10us in, sg1 ~7us later, and each ring carries only two
transfers (each extra dma_start on a ring costs ~1-3us of turnaround).
Weights pre-packed [128, 8, 192] ([Wk | Wq | Wv] per h-block, 1/8
folded into Wq) on the gpsimd ring.

Compute is pipelined over four 512-column groups (group g gated on
super-group g//2):
  Packed projection [Wk | Wq] -> psum (k rows 0..63, q rows 64..127),
  ONE full-width [128,512] cast into kqT, q DMA-shifted to partitions
  0..63 on the gpsimd ring (so the sync ring is never head-of-line
  blocked by it). v proj -> vT cast -> 4 PE transposes into one psum
  tile -> one strided copy into v_aug natural rows with a ones column.
  Scores sT[kj, qi] per 128-key block into a [128,512] psum bank; exp
  on ScalarE (no max subtraction: scores bounded ~+-4); diagonal
  blocks masked with gpsimd affine_select; pT bf16.
  AV: oT[d, qi] += [v[kb] | ones].T @ pT[kb] accumulating into ONE
  psum bank per group; the ones column accumulates the softmax
  denominator in row 64 for free.
  Normalize per group with no SBUF<->SBUF DMA hops: DVE reciprocal
  reads the psum sums row [1,512] directly, gpsimd partition_broadcast
  fans it out, DVE multiply, DMA out (sync ring, after the inputs).

The schedule is one flat software-pipelined stream: 24 PE warm-up
matmuls cover sg0's DMA and ramp the HAM-gated PE clock (top tier
needs ~5us of continuous PE work and any idle drops it), then proj
g0/g1, scores running two ahead of AVs, with proj(g+1) slotted into
the score stream where its input has landed so the PE never stalls.
PSUM: 2 proj acc banks, 3 score banks, 2 oT banks, 1 v-transpose = 8.
"""

import sys
from contextlib import ExitStack

if "/opt/trn_rl_repo" not in sys.path:
    sys.path.insert(0, "/opt/trn_rl_repo")

import numpy as np
import ml_dtypes

import concourse.bass as bass
import concourse.tile as tile
from concourse import bacc, mybir
from concourse.bass_utils import run_bass_kernel_spmd

B, T, H, D = 8, 2048, 1024, 64
N_CORES = 8
HB = H // 128  # 8 h-blocks
G = 4  # 512-wide column groups
GW = T // G  # 512
KB = T // 128  # 16 key blocks
KPG = KB // G  # 4 key blocks per group
SG = 2  # input super-groups (two 1024-wide DMA waves)

LINEARIZE = False
F32 = mybir.dt.float32
BF16 = mybir.dt.bfloat16


def build_kernel():
    nc = bacc.Bacc("TRN2", target_bir_lowering=False, debug=False, num_devices=N_CORES)

    # xt: [sg, partition-half, 64, h-block, t]; w: [128, hb, 192] = [Wk|Wq|Wv]
    xt_d = nc.dram_tensor(
        "xt", [SG, 2, 64, HB, T // SG], BF16, kind="ExternalInput"
    ).ap()
    w_d = nc.dram_tensor("w", [128, HB, 3 * D], BF16, kind="ExternalInput").ap()
    out_d = nc.dram_tensor("out", [D, T], F32, kind="ExternalOutput").ap()

    with tile.TileContext(nc, linearize=LINEARIZE) as tc:
        _build(tc, xt_d, w_d, out_d)

    nc.compile()
    return nc


def _build(tc, xt_d, w_d, out_d):
    nc = tc.nc
    ctx = ExitStack()
    singles = ctx.enter_context(tc.tile_pool(name="singles", bufs=1))
    pspool = ctx.enter_context(tc.tile_pool(name="pspool", bufs=2, space="PSUM"))
    scpool = ctx.enter_context(tc.tile_pool(name="scpool", bufs=3, space="PSUM"))
    opool = ctx.enter_context(tc.tile_pool(name="opool", bufs=2, space="PSUM"))
    vtpool = ctx.enter_context(tc.tile_pool(name="vtpool", bufs=1, space="PSUM"))
    ppool = ctx.enter_context(tc.tile_pool(name="ppool", bufs=4))

    # ---- input DMAs: two waves, each spread across both rings ----
    w_s = singles.tile([128, HB, 3 * D], BF16)
    xt_s = singles.tile([128, SG, HB, T // SG], BF16)
    wu_s = singles.tile([128, 512], BF16, name="wu_s")
    nc.vector.memset(wu_s[:], 0.0)
    nc.gpsimd.dma_start(out=w_s[:], in_=w_d[:])
    for sg in range(SG):
        nc.sync.dma_start(
            out=xt_s[0:64, sg].rearrange("p hb t -> p (hb t)"),
            in_=xt_d[sg, 0].rearrange("p hb t -> p (hb t)"),
        )
        nc.scalar.dma_start(
            out=xt_s[64:128, sg].rearrange("p hb t -> p (hb t)"),
            in_=xt_d[sg, 1].rearrange("p hb t -> p (hb t)"),
        )

    wkq = w_s[:, :, 0:128]  # [Wk | Wq] stationary halves
    wv = w_s[:, :, 128:192]

    kqT = singles.tile([128, T], BF16)  # rows 0..63 kT, rows 64..127 q
    qlo = singles.tile([64, T], BF16)  # q DMA-shifted to partitions 0..63
    vT = singles.tile([64, T], BF16)

    v_aug = singles.tile([128, KB, 65], BF16)
    nc.gpsimd.memset(v_aug[:, :, 64:65], 1.0)
    identb = singles.tile([64, 64], BF16)
    nc.gpsimd.memset(identb[:], 0.0)
    nc.gpsimd.affine_select(
        out=identb[:], in_=identb[:], compare_op=mybir.AluOpType.not_equal,
        fill=1.0, base=0, pattern=[[-1, 64]], channel_multiplier=1,
    )

    oT_s = singles.tile([64, T], F32)
    pt = {}  # (g, kb) -> bf16 tile
    oT_b = {}  # g -> [65, 512] psum tile

    def xg(g, hb):
        # xt columns of group g: half of super-group g//2
        return xt_s[:, g // 2, hb, bass.ds((g % 2) * GW, GW)]

    def emit_warmup(n):
        wu_ps = pspool.tile([128, 512], F32, tag="ps", name="warmup")
        for _ in range(n):
            nc.tensor.matmul(
                wu_ps[:], wu_s[:, 0:128], wu_s[:], start=True, stop=True
            )

    def emit_proj_kq(g):
        gcols = bass.ds(g * GW, GW)
        acc = pspool.tile([128, GW], F32, tag="ps", name=f"acc_kq_{g}")
        for hb in range(HB):
            nc.tensor.matmul(
                acc[:], wkq[:, hb, :], xg(g, hb),
                start=(hb == 0), stop=(hb == HB - 1),
            )
        nc.vector.tensor_copy(kqT[:, gcols], acc[:])
        nc.gpsimd.dma_start(out=qlo[:, gcols], in_=kqT[64:128, gcols])

    def emit_proj_v(g):
        gcols = bass.ds(g * GW, GW)
        acc = pspool.tile([64, GW], F32, tag="ps", name=f"acc_v_{g}")
        for hb in range(HB):
            nc.tensor.matmul(
                acc[:], wv[:, hb, :], xg(g, hb),
                start=(hb == 0), stop=(hb == HB - 1),
            )
        nc.vector.tensor_copy(vT[:, gcols], acc[:])
        # natural v rows via PE transpose: 4 blocks into one psum tile,
        # then one strided copy into v_aug
        vtr = vtpool.tile([128, KPG, 64], BF16, tag="vt", name=f"vtr_{g}")
        for j in range(KPG):
            kb = KPG * g + j
            nc.tensor.transpose(vtr[:, j], vT[:, bass.ts(kb, 128)], identb[:])
        nc.vector.tensor_copy(v_aug[:, KPG * g : KPG * (g + 1), 0:64], vtr[:])

    def emit_score(g, kb):
        # score block: kj in [kb*128, kb*128+128), qi in group g cols
        l0 = max(kb * 128 - g * GW, 0)  # group-local first column
        s_ps = scpool.tile([128, GW], F32, tag="sc", name=f"s_{g}_{kb}")
        nc.tensor.matmul(
            s_ps[:, l0:GW],
            kqT[0:64, bass.ts(kb, 128)],
            qlo[:, bass.ds(g * GW + l0, GW - l0)],
            start=True,
            stop=True,
        )
        p = ppool.tile([128, GW], BF16, tag="pt", name=f"pt_{g}_{kb}")
        pt[(g, kb)] = p
        nc.scalar.activation(
            out=p[:, l0:GW],
            in_=s_ps[:, l0:GW],
            func=mybir.ActivationFunctionType.Exp,
        )
        if kb >= KPG * g:
            # diagonal block: zero where kj (partition) > qi (free)
            nc.gpsimd.affine_select(
                out=p[:, l0 : l0 + 128],
                in_=p[:, l0 : l0 + 128],
                compare_op=mybir.AluOpType.is_ge,
                fill=0.0,
                base=0,
                pattern=[[1, 128]],
                channel_multiplier=-1,
            )

    def emit_av(g, kb):
        l0 = max(kb * 128 - g * GW, 0)
        if g not in oT_b:
            oT_b[g] = opool.tile([65, GW], F32, tag="o", name=f"oT_{g}")
        nc.tensor.matmul(
            oT_b[g][:, l0:GW],
            v_aug[:, kb, :],
            pt[(g, kb)][:, l0:GW],
            start=(kb == 0),
            stop=(kb == KPG * (g + 1) - 1),
        )

    def emit_norm(g):
        # normalize + store group g: psum sums row -> DVE reciprocal ->
        # gpsimd partition_broadcast -> DVE multiply -> DMA out.
        gcols = bass.ds(g * GW, GW)
        rcp = singles.tile([1, GW], F32, name=f"rcp_{g}")
        nc.vector.reciprocal(out=rcp[:], in_=oT_b[g][64:65, :])
        rb = singles.tile([64, GW], F32, name=f"rb_{g}")
        nc.gpsimd.partition_broadcast(rb[:], rcp[:])
        nc.vector.tensor_mul(oT_s[:, gcols], oT_b[g][0:64, :], rb[:])
        nc.sync.dma_start(out=out_d[:, gcols], in_=oT_s[:, gcols])

    # ---- flat schedule ----
    # proj slots are placed where their super-group's DMA has landed;
    # scores run two ahead of AVs so exp (ScalarE) is always overlapped.
    emit_warmup(24)
    emit_proj_kq(0)
    emit_proj_v(0)
    emit_proj_kq(1)
    emit_proj_v(1)
    # g0: kbs 0..3 (all diagonal-clipped)
    emit_score(0, 0)
    emit_score(0, 1)
    emit_score(0, 2)
    emit_av(0, 0)
    emit_score(0, 3)
    emit_av(0, 1)
    emit_av(0, 2)
    emit_av(0, 3)
    emit_norm(0)
    # g1: kbs 0..7; proj(2) slotted once sg1 has landed (~+17us)
    emit_score(1, 0)
    emit_score(1, 1)
    emit_score(1, 2)
    emit_av(1, 0)
    emit_score(1, 3)
    emit_av(1, 1)
    emit_score(1, 4)
    emit_av(1, 2)
    emit_score(1, 5)
    emit_av(1, 3)
    emit_proj_kq(2)
    emit_score(1, 6)
    emit_av(1, 4)
    emit_score(1, 7)
    emit_av(1, 5)
    emit_proj_v(2)
    emit_av(1, 6)
    emit_av(1, 7)
    emit_norm(1)
    # g2: kbs 0..11; proj(3) slotted mid-stream
    emit_score(2, 0)
    emit_score(2, 1)
    emit_score(2, 2)
    emit_av(2, 0)
    emit_score(2, 3)
    emit_av(2, 1)
    emit_score(2, 4)
    emit_av(2, 2)
    emit_proj_kq(3)
    emit_score(2, 5)
    emit_av(2, 3)
    emit_score(2, 6)
    emit_av(2, 4)
    emit_score(2, 7)
    emit_av(2, 5)
    emit_proj_v(3)
    emit_score(2, 8)
    emit_av(2, 6)
    emit_score(2, 9)
    emit_av(2, 7)
    emit_score(2, 10)
    emit_av(2, 8)
    emit_score(2, 11)
    emit_av(2, 9)
    emit_av(2, 10)
    emit_av(2, 11)
    emit_norm(2)
    # g3: kbs 0..15
    emit_score(3, 0)
    emit_score(3, 1)
    for kb in range(2, KB):
        emit_score(3, kb)
        emit_av(3, kb - 2)
    emit_av(3, KB - 2)
    emit_av(3, KB - 1)
    emit_norm(3)

    ctx.close()


_NC_CACHE = {}


def _get_nc():
    if "nc" not in _NC_CACHE:
        _NC_CACHE["nc"] = build_kernel()
    return _NC_CACHE["nc"]


def make_in_maps(x, Wk, Wq, Wv):
    bf16 = ml_dtypes.bfloat16
    x = np.asarray(x, dtype=np.float32)
    wq = np.asarray(Wq, dtype=np.float32) / np.sqrt(np.float32(D))
    wk = np.asarray(Wk, dtype=np.float32)
    wv = np.asarray(Wv, dtype=np.float32)
    # [H, 192] = [Wk | Wq | Wv], then -> [128, HB, 192] (h = hb*128 + p)
    w = np.concatenate([wk, wq, wv], axis=1).astype(bf16)
    w = np.ascontiguousarray(w.reshape(HB, 128, 3 * D).transpose(1, 0, 2))
    in_maps = []
    for b in range(B):
        xt = x[b].T.astype(bf16)  # [H, T]
        # [SG, 2, 64, HB, T//SG]: per (sg, half, partition) one 16KB run
        xt = np.ascontiguousarray(
            xt.reshape(HB, 2, 64, SG, T // SG).transpose(3, 1, 2, 0, 4)
        )
        in_maps.append({"xt": xt, "w": w})
    return in_maps


def kernel(x, Wk, Wq, Wv, **_ignored):
    nc = _get_nc()
    in_maps = make_in_maps(x, Wk, Wq, Wv)
    res = run_bass_kernel_spmd(nc, in_maps, core_ids=list(range(N_CORES)))
    out = np.stack([res.results[b]["out"].T for b in range(B)])
    return out.astype(np.float32)


if __name__ == "__main__":
    x = np.random.randn(B, T, H).astype(np.float32)
    s = 1.0 / np.sqrt(H)
    Wk = np.random.uniform(-s, s, (H, D)).astype(np.float32)
    Wq = np.random.uniform(-s, s, (H, D)).astype(np.float32)
    Wv = np.random.uniform(-s, s, (H, D)).astype(np.float32)
    out = kernel(x=x, Wk=Wk, Wq=Wq, Wv=Wv)
    print("out shape:", out.shape, "finite:", np.isfinite(out).all())
